# revision 6
# baseline (speedup 1.0000x reference)
"""Trainium2 Bass kernel for Autoformer-style autocorrelation attention.

Math (matches the reference nn.Module):
    top_k = int(log(L)) = 6
    mean_value[b, l] = corr[b].mean(over H, C)                     # [B, L]
    idx = top_k(mean_value.mean(over B))                           # [6]
    w = softmax(mean_value[:, idx], axis=-1)                       # [B, 6]
    out[b, h, c, l] = sum_k w[b, k] * values[b, h, c, (l+idx_k)%L]

Strategy: data-parallel over B (4 batches per core on 8 cores), two
launches with tiny host glue (top-k + softmax) in between.

Both launches use a host-permuted "P-major" DRAM layout
buf[b, p, t, l] = x[b, t*128 + p, l] so each SBUF partition's slice of a
batch is one contiguous 8 KiB run in DRAM -> 128 large DMA packets per
batch instead of 512x 2 KiB, keeping the DMA engines at peak rate with
one descriptor per partition.

Launch 1 reduces corr (fp16; the ~1e-5 quantization error on the means
is far below the 1.1e-4 top-k selection margin, fp8/int8 are NOT safe)
over (H, C) per batch via ones-matmuls: psum[l] += ones[128].corr_tile
accumulated over the 4 row-tiles of the batch.  No warmup junk, no
scalar-engine use (skips the ACT table load), minimal instruction count
so the framework's semaphore epilogue stays short.  [B, L] sums return
to host, which runs the tiny top-k + softmax glue.

Launch 2 computes the 6-shift weighted combine with the shifts baked in
as static SBUF column windows, splitting the 6 terms across engines by
measured throughput (PE 2.4 cols/ns, DVE ~1 col/ns fp16 2x-mode, Sc
~0.6 cols/ns but closest to PSUM):
  - terms 0-3: tensor engine, diag(w_bk) @ shifted-window matmuls
    accumulating in PSUM (fp16, 1 col/cycle),
  - PSUM -> fp16 SBUF conversion: scalar engine ACTIVATE copies,
  - terms 4-5: vector engine scalar_tensor_tensor chain (out =
    w_k*shift_k(v) + acc, all-16-bit so the 2x DVE mode engages,
    per-batch weights enter as [128,1] AP scalars from an input tensor
    so one compiled NEFF is SPMD across all 8 cores),
  - output is written fp16 (host upcasts to fp32; adds ~5e-4 relative,
    tolerance is 2e-2) halving the output DMA traffic.
"""

import math

import numpy as np

_B, _H, _C, _L = 32, 8, 64, 1024
_NCORES = 8
_BLOC = _B // _NCORES  # batches per core
_R = _H * _C           # rows per batch
_PART = 128
_TPB = _R // _PART     # SBUF row-tiles per batch (4)
_TOPK = int(math.log(_L))  # 6
_NPE = 4               # shift terms handled by the tensor engine
_NDVE = _TOPK - _NPE   # shift terms handled by the vector engine (2)
_HALF = 512            # PSUM bank width in fp32


def _build_phase1():
    import concourse.bacc as bacc
    import concourse.mybir as mybir
    import concourse.tile as tile

    f32 = mybir.dt.float32
    f16 = mybir.dt.float16
    nc = bacc.Bacc("TRN2", target_bir_lowering=False, debug=False,
                   enable_partition_id=False)
    # P-major: corr_sh[b, p, t*L + l] = corr[b, t*128 + p, l]
    corr_d = nc.dram_tensor("corr_sh", [_BLOC, _PART, _TPB * _L], f16,
                            kind="ExternalInput").ap()
    sums_d = nc.dram_tensor("sums", [1, _BLOC * _L], f32,
                            kind="ExternalOutput").ap()

    with tile.TileContext(nc) as tc:
        with (
            tc.tile_pool(name="io", bufs=2) as io_pool,
            tc.tile_pool(name="const", bufs=1) as const_pool,
            tc.tile_pool(name="acc", bufs=1) as acc_pool,
            tc.tile_pool(name="ps", bufs=2, space="PSUM") as ps_pool,
        ):
            # Prefetch the first two batches before anything else so the
            # DMA engines start streaming during the framework preamble.
            vts = []
            for b in range(min(2, _BLOC)):
                vt = io_pool.tile([_PART, _TPB * _L], f16, tag="vt")
                nc.sync.dma_start(vt[:], corr_d[b])
                vts.append(vt)
            ones = const_pool.tile([_PART, _PART], f16)
            nc.vector.memset(ones[:], 1.0)
            outs = acc_pool.tile([1, _BLOC * _L], f32)

            for b in range(_BLOC):
                if b < 2:
                    vt = vts[b]
                else:
                    vt = io_pool.tile([_PART, _TPB * _L], f16, tag="vt")
                    nc.sync.dma_start(vt[:], corr_d[b])
                pss = [ps_pool.tile([_PART, _HALF], f32, tag=f"ps{h}",
                                    name=f"ps{b}_{h}")
                       for h in range(2)]
                for t in range(_TPB):
                    for h in range(2):
                        nc.tensor.matmul(
                            pss[h][:],
                            ones[:],
                            vt[:, t * _L + h * _HALF:t * _L + (h + 1) * _HALF],
                            start=(t == 0),
                            stop=(t == _TPB - 1),
                        )
                for h in range(2):
                    o0 = b * _L + h * _HALF
                    nc.vector.tensor_copy(outs[0:1, o0:o0 + _HALF],
                                          pss[h][0:1, :])
            nc.sync.dma_start(sums_d[0:1, :], outs[0:1, :])
    nc.compile()
    return nc


def _build_phase2(idx):
    import concourse.bacc as bacc
    import concourse.mybir as mybir
    import concourse.tile as tile

    f16 = mybir.dt.float16
    f32 = mybir.dt.float32
    alu = mybir.AluOpType
    act_copy = mybir.ActivationFunctionType.Copy

    nc = bacc.Bacc("TRN2", target_bir_lowering=False, debug=False,
                   enable_partition_id=False)
    # P-major layout (see module docstring)
    vals_d = nc.dram_tensor("vals", [_BLOC, _PART, _TPB * _L], f16,
                            kind="ExternalInput").ap()
    # per-batch weights for the DVE terms, broadcast across partitions
    wsb_d = nc.dram_tensor("wsb", [_PART, _BLOC * _TOPK], f16,
                           kind="ExternalInput").ap()
    # diag(w[b,k]) blocks for the PE terms
    diag_d = nc.dram_tensor("diags", [_PART, _BLOC * _NPE * _PART], f16,
                            kind="ExternalInput").ap()
    out_d = nc.dram_tensor("out_sh", [_BLOC, _PART, _TPB * _L], f16,
                           kind="ExternalOutput").ap()

    with tile.TileContext(nc) as tc:
        with (
            tc.tile_pool(name="const", bufs=1) as const_pool,
            tc.tile_pool(name="v16", bufs=2) as v16_pool,
            tc.tile_pool(name="tmp", bufs=4) as tmp_pool,
            tc.tile_pool(name="out", bufs=2) as out_pool,
            tc.tile_pool(name="ps", bufs=4, space="PSUM") as ps_pool,
        ):
            vts = []
            for b in range(min(2, _BLOC)):
                vt = v16_pool.tile([_PART, _TPB * _L], f16, tag="vt")
                nc.sync.dma_start(vt[:], vals_d[b])
                vts.append(vt)
            w_t = const_pool.tile([_PART, _BLOC * _TOPK], f16)
            nc.sync.dma_start(w_t[:], wsb_d[:])
            diag = const_pool.tile([_PART, _BLOC * _NPE * _PART], f16)
            nc.sync.dma_start(diag[:], diag_d[:])

            for b in range(_BLOC):
                if b < 2:
                    vt = vts[b]
                else:
                    vt = v16_pool.tile([_PART, _TPB * _L], f16, tag="vt")
                    nc.sync.dma_start(vt[:], vals_d[b])
                ot = out_pool.tile([_PART, _TPB * _L], f16, tag="ot")
                for t in range(_TPB):
                    c0 = t * _L  # this tile's column window in vt/ot

                    # --- PE: terms 0..3 accumulate into 2 PSUM halves ---
                    pss = [ps_pool.tile([_PART, _HALF], f32, tag=f"ps{h}",
                                        name=f"ps{h}")
                           for h in range(2)]
                    pieces = {0: [], 1: []}
                    for k in range(_NPE):
                        dof = (b * _NPE + k) * _PART
                        for h in range(2):
                            s = (idx[k] + h * _HALF) % _L
                            n1 = min(_HALF, _L - s)
                            pieces[h].append((dof, 0, n1, s))
                            if n1 < _HALF:
                                pieces[h].append((dof, n1, _HALF, 0))
                    for h in range(2):
                        for pi, (dof, o0, o1, s) in enumerate(pieces[h]):
                            nc.tensor.matmul(
                                pss[h][:, o0:o1],
                                diag[:, dof:dof + _PART],
                                vt[:, c0 + s:c0 + s + (o1 - o0)],
                                start=(pi == 0),
                                stop=(pi == len(pieces[h]) - 1),
                            )

                    # --- Scalar: PSUM -> fp16 SBUF conversion ---
                    x1 = tmp_pool.tile([_PART, _L], f16, tag="x1")
                    for h in range(2):
                        nc.scalar.activation(
                            x1[:, h * _HALF:(h + 1) * _HALF], pss[h][:],
                            act_copy)

                    # --- DVE: terms 4..5 fused accumulate chain ---
                    x2 = tmp_pool.tile([_PART, _L], f16, tag="x2")
                    stages = [(idx[_NPE], x1, x2), (idx[_NPE + 1], x2, None)]
                    for j, (s, src, dst) in enumerate(stages):
                        wap = w_t[:, b * _TOPK + _NPE + j:
                                  b * _TOPK + _NPE + j + 1]
                        dtile = ot if dst is None else dst
                        dof = c0 if dst is None else 0
                        # piece 1: out[0:L-s] = w*v[s:L] + src[0:L-s]
                        n1 = _L - s
                        if n1:
                            nc.vector.scalar_tensor_tensor(
                                dtile[:, dof:dof + n1],
                                vt[:, c0 + s:c0 + _L],
                                wap,
                                src[:, 0:n1],
                                op0=alu.mult, op1=alu.add)
                        if s:
                            nc.vector.scalar_tensor_tensor(
                                dtile[:, dof + n1:dof + _L],
                                vt[:, c0:c0 + s],
                                wap,
                                src[:, n1:_L],
                                op0=alu.mult, op1=alu.add)
                nc.scalar.dma_start(out_d[b], ot[:])
    nc.compile()
    return nc


def _run_spmd(nc, in_maps, **kwargs):
    from concourse import bass_utils

    return bass_utils.run_bass_kernel_spmd(
        nc, in_maps, core_ids=list(range(_NCORES)), **kwargs
    )


def _pmajor(x):
    """[n, R, L] -> [n, 128, TPB*L] with buf[n, p, t*L+l] = x[n, t*128+p, l]."""
    n = x.shape[0]
    return np.ascontiguousarray(
        x.reshape(n, _TPB, _PART, _L).transpose(0, 2, 1, 3)
        .reshape(n, _PART, _TPB * _L))


def _unpmajor(x):
    """Inverse of _pmajor."""
    n = x.shape[0]
    return (x.reshape(n, _PART, _TPB, _L).transpose(0, 2, 1, 3)
            .reshape(n, _R, _L))


def kernel(values: np.ndarray, corr: np.ndarray, _collect=None) -> np.ndarray:
    assert values.shape == (_B, _H, _C, _L) and corr.shape == (_B, _H, _C, _L)
    corr16 = _pmajor(np.asarray(corr, dtype=np.float16).reshape(_B, _R, _L))
    vals16 = _pmajor(np.asarray(values, dtype=np.float16).reshape(_B, _R, _L))

    # ---- launch 1: per-batch sums of corr over (H, C) ----
    nc1 = _build_phase1()
    in1 = [
        {"corr_sh": corr16[c * _BLOC:(c + 1) * _BLOC]}
        for c in range(_NCORES)
    ]
    res1 = _run_spmd(nc1, in1, **(_collect.kwargs(1) if _collect else {}))
    if _collect is not None:
        _collect.add(1, nc1, res1)
    sums = np.concatenate(
        [r["sums"].reshape(_BLOC, _L) for r in res1.results], axis=0
    )  # [B, L]

    # ---- host glue: top-k indices + softmax weights (tiny) ----
    mean_value = sums / np.float32(_R)                       # [B, L]
    g = mean_value.astype(np.float64).mean(axis=0)           # [L]
    idx = np.argsort(-g, kind="stable")[:_TOPK].astype(np.int64)
    wsel = mean_value[:, idx].astype(np.float32)             # [B, 6]
    e = np.exp(wsel - wsel.max(axis=-1, keepdims=True))
    w = (e / e.sum(axis=-1, keepdims=True)).astype(np.float32)

    # ---- launch 2: weighted shifted-gather combine ----
    nc2 = _build_phase2([int(i) for i in idx])
    eye = np.eye(_PART, dtype=np.float16)
    in2 = []
    for c in range(_NCORES):
        wloc = w[c * _BLOC:(c + 1) * _BLOC]                  # [BLOC, 6]
        wsb = np.ascontiguousarray(
            np.broadcast_to(
                wloc.reshape(-1)[None, :], (_PART, _BLOC * _TOPK)),
            dtype=np.float16,
        )
        diags = np.concatenate(
            [eye * np.float16(wloc[b, k]) for b in range(_BLOC)
             for k in range(_NPE)],
            axis=1,
        )  # [128, BLOC*NPE*128] fp16
        in2.append({
            "vals": vals16[c * _BLOC:(c + 1) * _BLOC],
            "wsb": wsb,
            "diags": np.ascontiguousarray(diags),
        })
    res2 = _run_spmd(nc2, in2, **(_collect.kwargs(2) if _collect else {}))
    if _collect is not None:
        _collect.add(2, nc2, res2)
    out = np.concatenate([_unpmajor(r["out_sh"]) for r in res2.results],
                         axis=0).astype(np.float32)
    return out.reshape(_B, _H, _C, _L)
